# revision 1
# baseline (speedup 1.0000x reference)
"""Trainium2 Bass kernel for nn_AttentionStem (5x5 local attention stem, stride 2).

Self-contained: hardcodes shapes B=8, CIN=64, H=W=128, OUT_CH=128, M=2, K=5.
Data-parallel over batch: one batch element per NeuronCore (8 cores).

Math (per batch):
  scores[k,(h,w)] = x_s(2h,2w)^T G x(p'_k),  G = w_q^T w_k   (q/k projections folded)
  attn = softmax_k(scores)
  out[c,(h,w)] = sum_k attn_k sum_m wpos[m,k] v[2c+m, p'_k],  v = w_v x

Layout tricks vs v1:
  - x stored bf16 with even image rows on partitions 0:64 and odd rows on
    64:128 -> V/ST matmuls (K=64) run as two concurrent PE row-tiles.
  - y duplicated on both partition halves via lhsT=[G|G].
  - score slabs trimmed to 896 cols (t=1 row only feeds one pair).
  - elementwise work split between ACT and DVE.
"""

import os
import sys

for _p in ("/opt/pypackages", "/opt/trn_rl_repo"):
    if _p not in sys.path:
        sys.path.insert(0, _p)

from contextlib import ExitStack

import ml_dtypes
import numpy as np

import concourse.bacc as bacc
import concourse.bass as bass
import concourse.mybir as mybir
from concourse.bass_utils import run_bass_kernel_spmd
from concourse.tile import TileContext

F32 = mybir.dt.float32
BF16 = mybir.dt.bfloat16

NCORES = 8
CIN = 64
IMG = 128          # input H = W
OC = 128           # out channels
VCH = 258          # V row pitch: 256 v-channels + 2 ones columns
VROWS = 132        # 2 pad + 128 + 2 pad rows in v_sb
HO = 64            # output H = W
NPAIR = 32         # output row pairs
SLABW = 896        # trimmed transposed-score slab width

# d (= key row r - 4j for pair j) -> col offset of its 128-col block in a slab.
# Blocks from even key rows (PE row-tile 0) sit in PSUM bank A (cols 0:512),
# odd-row blocks (tile 1) in bank B (cols 512:896) — concurrent row-tiled
# matmuls writing the same PSUM bank hang TRN2.
OFF_OF_D = {4: 0, 0: 128, 2: 256, -2: 384, 1: 512, 3: 640, -1: 768}

APPLY_COLSPLIT = os.environ.get("APPLY_COLSPLIT", "0") == "1"


def make_wpos(row_emb, col_emb, mix_emb):
    a = mix_emb.T.astype(np.float64) @ row_emb.astype(np.float64)  # [2,5]
    b = mix_emb.T.astype(np.float64) @ col_emb.astype(np.float64)  # [2,5]
    wp = a[:, :, None] + b[:, None, :]                             # [2,5,5]
    wp = wp - wp.max(axis=0, keepdims=True)
    e = np.exp(wp)
    wp = e / e.sum(axis=0, keepdims=True)
    return wp.reshape(2, 25).astype(np.float32)                    # [m, dh*5+dw]


def make_masks(wpos):
    """wpos-weighted band masks in the trimmed ST layout.

    Returns [128 (kcol), 2 (m), 896] f32; block at OFF_OF_D[d] holds the
    masks for key row r = 4j + d of pair j, cols rho*64 + w."""
    wm = np.zeros((128, 2, SLABW), np.float32)
    for d, base in OFF_OF_D.items():
        for rho in (0, 1):
            dh = d + 2 - 2 * rho
            if not 0 <= dh < 5:
                continue
            for w in range(64):
                for dw in range(5):
                    kc = 2 * w + dw - 2
                    if 0 <= kc < 128:
                        wm[kc, :, base + rho * 64 + w] = wpos[:, dh * 5 + dw]
    return wm


def make_oob():
    """#window entries with out-of-image column, per position in a pair."""
    oob = np.zeros((128, 1), np.float32)
    for rho in (0, 1):
        for w in range(64):
            cnt = sum(1 for dw in range(5) if not 0 <= 2 * w + dw - 2 < 128)
            oob[rho * 64 + w, 0] = 5.0 * cnt
    return oob


def _ap(t, off, dims, p0=0, pn=None):
    a = t[:]
    np_ = pn if pn is not None else a.ap[0][1]
    return bass.AP(tensor=a.tensor, offset=off + p0 * a.ap[0][0],
                   ap=[[a.ap[0][0], np_]] + [list(d) for d in dims])


def build_nc():
    nc = bacc.Bacc("TRN2", target_bir_lowering=False, debug=False, num_devices=NCORES)

    xe_d = nc.dram_tensor("xe", [CIN, 64, IMG], BF16, kind="ExternalInput")
    xo_d = nc.dram_tensor("xo", [CIN, 64, IMG], BF16, kind="ExternalInput")
    g2_d = nc.dram_tensor("g2", [CIN, 128], BF16, kind="ExternalInput")
    wvt_d = nc.dram_tensor("wvt", [128, 256], BF16, kind="ExternalInput")
    wm_d = nc.dram_tensor("wmask", [128, 2 * SLABW], BF16, kind="ExternalInput")
    oob_d = nc.dram_tensor("oob", [128, 1], F32, kind="ExternalInput")
    out_d = nc.dram_tensor("out", [HO * HO, OC], F32, kind="ExternalOutput")

    EXP = mybir.ActivationFunctionType.Exp
    COPY = mybir.ActivationFunctionType.Copy

    with TileContext(nc) as tc, ExitStack() as ctx:
        sg = ctx.enter_context(tc.tile_pool(name="singles", bufs=1))
        # x: partitions 0:64 even image rows, 64:128 odd rows; 64 rows x 128 cols
        x_sb = sg.tile([128, 64 * IMG], BF16)
        v_sb = sg.tile([128, VROWS * VCH], BF16)     # V + ones cols, padded rows
        y_sb = sg.tile([128, 4096], BF16)            # queries, dup on both halves
        wm_sb = sg.tile([128, 2 * SLABW], BF16)
        oob_sb = sg.tile([128, 1], F32)
        g2_sb = sg.tile([64, 128], BF16)
        wvt_sb = sg.tile([128, 256], BF16)

        # g2 first (gates the y prologue); bulky constants after the x load
        nc.sync.dma_start(out=g2_sb[:], in_=g2_d.ap())

        # x load: 8 chunks of 8 packed rows per half, spread over 3 queues
        qs = [nc.gpsimd, nc.sync, nc.scalar]
        for c8 in range(8):
            dst_e = _ap(x_sb, c8 * 8 * IMG, [[1, 8 * IMG]], 0, 64)
            dst_o = _ap(x_sb, c8 * 8 * IMG, [[1, 8 * IMG]], 64, 64)
            src_e = xe_d.ap()[:, c8 * 8:(c8 + 1) * 8, :]
            src_o = xo_d.ap()[:, c8 * 8:(c8 + 1) * 8, :]
            qs[(2 * c8) % 3].dma_start(out=dst_e, in_=src_e)
            qs[(2 * c8 + 1) % 3].dma_start(out=dst_o, in_=src_o)

        nc.sync.dma_start(out=wvt_sb[:], in_=wvt_d.ap())
        nc.sync.dma_start(out=wm_sb[:], in_=wm_d.ap())
        nc.sync.dma_start(out=oob_sb[:], in_=oob_d.ap())

        # V pad rows (zero) + ones columns
        nc.vector.memset(_ap(v_sb, 0, [[1, 2 * VCH]]), 0.0)
        nc.vector.memset(_ap(v_sb, 130 * VCH, [[1, 2 * VCH]]), 0.0)
        nc.vector.memset(_ap(v_sb, 256, [[VCH, VROWS], [1, 2]]), 1.0)

        def xrow(r):
            # key row r: [64 partitions (channels), 128 cols] on its parity half
            p = (r & 1) * 64
            return x_sb[p:p + 64, (r >> 1) * IMG:(r >> 1) * IMG + IMG]

        # ---- fused pipeline: y prologue, then V(s+3) + ST(s) + apply(s-1)
        # per loop iteration, keeping the PE queue dense (HAM stays warm).
        with tc.tile_pool(name="big", bufs=3, space="PSUM") as big, \
             tc.tile_pool(name="aps", bufs=2, space="PSUM") as aps, \
             tc.tile_pool(name="e2t", bufs=4) as e2t, \
             tc.tile_pool(name="a0p", bufs=5) as a0p, \
             tc.tile_pool(name="a1p", bufs=5) as a1p, \
             tc.tile_pool(name="outsb", bufs=4) as outsb, \
             tc.tile_pool(name="dens", bufs=6) as dens:

            A = {}

            def make_y(ch):
                yp = big.tile([128, 1024], F32, tag="big")
                for i in range(2):
                    hs0 = ch * 16 + i * 8
                    rhs = _ap(x_sb, hs0 * IMG, [[IMG, 8], [2, 64]], 0, 64)
                    nc.tensor.matmul(yp[:, i * 512:(i + 1) * 512], g2_sb[:],
                                     rhs, start=True, stop=True)
                if ch % 2 == 0:
                    nc.scalar.copy(y_sb[:, ch * 1024:(ch + 1) * 1024], yp[:])
                else:
                    nc.vector.tensor_copy(y_sb[:, ch * 1024:(ch + 1) * 1024], yp[:])

            VP = {}

            def make_v_mms(vs):
                vp = big.tile([128, 1024], F32, tag="big")
                VP[vs] = vp
                # even rows -> bank A (cols 0:512), odd rows -> bank B:
                # concurrent row-tiles must not write the same PSUM bank.
                # Emit rows in order (alternating tiles) so pairs overlap.
                for r_i, q in ((0, 0), (1, 2), (2, 1), (3, 3)):
                    r = vs * 4 + r_i
                    h = (r_i & 1) * 64
                    nc.tensor.matmul(vp[:, q * 256:(q + 1) * 256], xrow(r),
                                     wvt_sb[h:h + 64, :], start=True, stop=True)
            def make_v_copy(vs):
                vp = VP.pop(vs)
                # dst rows in order (r0, r2, r1, r3) to match quarter layout
                dst = _ap(v_sb, (vs * 4 + 2) * VCH,
                          [[VCH, 2], [2 * VCH, 2], [1, 256]])
                src = vp[:].rearrange("p (r c) -> p r c", c=256)
                if vs % 2 == 0:
                    nc.scalar.copy(dst, src)
                else:
                    nc.vector.tensor_copy(dst, src)

            def make_slab(s):
                stp = big.tile([128, 1024], F32, tag="big")
                n = min(2, NPAIR - s) * 128
                # t=0 (tile0, bank A): pairs s-1, s -> cols 0:256
                if s == 0:
                    nc.tensor.matmul(stp[:, 128:256], xrow(0),
                                     y_sb[0:64, 0:128], start=True, stop=True)
                else:
                    nc.tensor.matmul(stp[:, 0:256], xrow(4 * s),
                                     y_sb[0:64, (s - 1) * 128:(s + 1) * 128],
                                     start=True, stop=True)
                # t=1 (tile1, bank B): pair s -> cols 512:640
                nc.tensor.matmul(stp[:, 512:640], xrow(4 * s + 1),
                                 y_sb[64:128, s * 128:(s + 1) * 128],
                                 start=True, stop=True)
                # t=2 (tile0, bank A): pairs s, s+1 -> cols 256:256+n
                nc.tensor.matmul(stp[:, 256:256 + n], xrow(4 * s + 2),
                                 y_sb[0:64, s * 128:s * 128 + n],
                                 start=True, stop=True)
                # t=3 (tile1, bank B): pairs s, s+1 -> cols 640:640+n
                nc.tensor.matmul(stp[:, 640:640 + n], xrow(4 * s + 3),
                                 y_sb[64:128, s * 128:s * 128 + n],
                                 start=True, stop=True)
                e2 = e2t.tile([128, SLABW], BF16)
                nc.scalar.activation(out=e2[:], in_=stp[:, 0:SLABW], func=EXP)
                a01 = a0p.tile([128, 2 * SLABW], BF16)
                e2r = bass.AP(tensor=e2[:].tensor, offset=e2[:].offset,
                              ap=[list(e2[:].ap[0]), [0, 2], [1, SLABW]])
                nc.vector.tensor_mul(a01[:], e2r, wm_sb[:])
                A[s] = (a01, a01)
                A.pop(s - 3, None)

            def apply_pair(j):
                ap_ps = aps.tile([128, 130], F32)
                ops = [(d, m) for d in (0, 1, 2, -2, -1, 3, 4) for m in (0, 1)]
                for idx, (d, m) in enumerate(ops):
                    r = 4 * j + d
                    off = OFF_OF_D[d]
                    if r < 0 or r >= IMG:
                        src = wm_sb
                    else:
                        src = A[r // 4][m]
                    off += m * SLABW
                    rhs = _ap(v_sb, (r + 2) * VCH + m, [[2, 129]])
                    st = idx == 0
                    sp = idx == len(ops) - 1
                    if APPLY_COLSPLIT:
                        nc.tensor.matmul(ap_ps[0:64, 0:129],
                                         src[:, off:off + 64], rhs,
                                         start=st, stop=sp,
                                         skip_group_check=True)
                        nc.tensor.matmul(ap_ps[64:128, 0:129],
                                         src[:, off + 64:off + 128], rhs,
                                         start=st, stop=sp,
                                         skip_group_check=True,
                                         tile_position=(0, 64))
                    else:
                        nc.tensor.matmul(ap_ps[:, 0:129],
                                         src[:, off:off + 128], rhs,
                                         start=st, stop=sp,
                                         skip_group_check=True)
                den = dens.tile([128, 1], F32)
                nc.vector.tensor_add(den[:], ap_ps[:, 128:129], oob_sb[:])
                rec = dens.tile([128, 1], F32)
                nc.vector.reciprocal(rec[:], den[:])
                o_sb = outsb.tile([128, 128], F32)
                nc.scalar.activation(out=o_sb[:], in_=ap_ps[:, 0:128],
                                     func=COPY, scale=rec[:])
                nc.gpsimd.dma_start(out=out_d.ap()[j * 128:(j + 1) * 128, :],
                                    in_=o_sb[:])

            for i in range(4):
                make_y(i)
                if i < 3:
                    make_v_mms(i)
                    make_v_copy(i)
            for s in range(NPAIR):
                make_slab(s)
                if s + 3 < NPAIR:
                    make_v_mms(s + 3)
                if s >= 1:
                    apply_pair(s - 1)
                if s + 3 < NPAIR:
                    make_v_copy(s + 3)
            apply_pair(NPAIR - 1)

    nc.compile()
    return nc


_NC_CACHE = None


def kernel(x, w_q, w_k, w_v, row_emb, col_emb, mix_emb):
    global _NC_CACHE
    x = np.asarray(x, np.float32)
    w_q = np.asarray(w_q, np.float32)
    w_k = np.asarray(w_k, np.float32)
    w_v = np.asarray(w_v, np.float32)
    row_emb = np.asarray(row_emb, np.float32)
    col_emb = np.asarray(col_emb, np.float32)
    mix_emb = np.asarray(mix_emb, np.float32)

    G = (w_q.T @ w_k).astype(ml_dtypes.bfloat16)          # [64, 64]
    g2 = np.hstack([G, G])                                # [64, 128]
    wvt = np.vstack([w_v.T] * 2).astype(ml_dtypes.bfloat16)  # [128, 256]
    wpos = make_wpos(row_emb, col_emb, mix_emb)
    wmask = make_masks(wpos).reshape(128, 2 * SLABW).astype(ml_dtypes.bfloat16)
    oob = make_oob()

    xb = x.astype(ml_dtypes.bfloat16)                      # [B, 64, 128, 128]
    xe = np.ascontiguousarray(xb[:, :, 0::2, :])           # [B, 64, 64, 128]
    xo = np.ascontiguousarray(xb[:, :, 1::2, :])

    if _NC_CACHE is None:
        _NC_CACHE = build_nc()
    nc = _NC_CACHE

    in_maps = []
    for b in range(NCORES):
        in_maps.append({
            "xe": xe[b],
            "xo": xo[b],
            "g2": g2,
            "wvt": wvt,
            "wmask": wmask,
            "oob": oob,
        })
    res = run_bass_kernel_spmd(nc, in_maps, core_ids=list(range(NCORES)))
    out = np.stack([res.results[b]["out"].T.reshape(OC, HO, HO) for b in range(NCORES)])
    return out.astype(np.float32)



# revision 4
# speedup vs baseline: 1.0934x; 1.0934x over previous
"""Trainium2 Bass kernel for nn_AttentionStem (5x5 local attention stem, stride 2).

Self-contained: hardcodes shapes B=8, CIN=64, H=W=128, OUT_CH=128, M=2, K=5.
Data-parallel over batch: one batch element per NeuronCore (8 cores).

Math (per batch):
  scores[k,(h,w)] = x_s(2h,2w)^T G x(p'_k),  G = w_q^T w_k   (q/k folded)
  a_m[key,pos]    = exp(score) * wpos_m(dh,dw) * band
  out[pos,ch]     = sum_m wv_m^T ( sum_d xTe_r(d)^T a_m ) / den   (V folded
                    through the apply: Q_m[c,pos] = sum_keys a_m x[c,key],
                    then out = wv_m^T Q_m -- the big V tensor is never
                    materialized, killing the PSUM->SBUF V copy.)

v2 layout:
  - x bf16, even image rows on partitions 0:64, odd on 64:128 (ST row-tiling).
  - xTe: x transposed to [imgcol, row, ch] + ones channel + 2 pad rows each
    side (host-prepared). stage-A lhsT; ones channel accumulates den in Q
    row 64; pad rows make OOB key rows contribute sum(wpos) to den only.
  - per-slab chain: ST mms -> ACT exp -> DVE mask -> (3 iters later)
    stage-A mms -> ACT Q-copy -> stage-B mms -> DVE den/recip/scale -> DMA.
  - out stored bf16, host casts to f32.
"""

import sys

for _p in ("/opt/pypackages", "/opt/trn_rl_repo"):
    if _p not in sys.path:
        sys.path.insert(0, _p)

from contextlib import ExitStack

import ml_dtypes
import numpy as np

import concourse.bacc as bacc
import concourse.bass as bass
import concourse.mybir as mybir
from concourse.bass_utils import run_bass_kernel_spmd
from concourse.tile import TileContext

F32 = mybir.dt.float32
BF16 = mybir.dt.bfloat16

NCORES = 8
CIN = 64
IMG = 128          # input H = W
OC = 128           # out channels
HO = 64            # output H = W
NPAIR = 32         # output row pairs
SLABW = 896        # trimmed transposed-score slab width
XTW = 65           # xTe row width: 64 channels + ones
XTROWS = 132       # 2 pad + 128 + 2 pad rows in xTe

# d (= key row r - 4j for pair j) -> col offset of its 128-col block in a slab.
# Blocks from even key rows (PE row-tile 0) sit in PSUM bank A (cols 0:512),
# odd-row blocks (tile 1) in bank B (cols 512:896) -- concurrent row-tiled
# matmuls writing the same PSUM bank hang TRN2.
OFF_OF_D = {4: 0, 0: 128, 2: 256, -2: 384, 1: 512, 3: 640, -1: 768}

DELAY_A = 3        # stage-A for pair s-DELAY_A is emitted in iteration s


def make_wpos(row_emb, col_emb, mix_emb):
    a = mix_emb.T.astype(np.float64) @ row_emb.astype(np.float64)  # [2,5]
    b = mix_emb.T.astype(np.float64) @ col_emb.astype(np.float64)  # [2,5]
    wp = a[:, :, None] + b[:, None, :]                             # [2,5,5]
    wp = wp - wp.max(axis=0, keepdims=True)
    e = np.exp(wp)
    wp = e / e.sum(axis=0, keepdims=True)
    return wp.reshape(2, 25).astype(np.float32)                    # [m, dh*5+dw]


def make_masks(wpos):
    """wpos-weighted band masks in the trimmed ST layout.

    Returns [128 (kcol), 2 (m), 896] f32; block at OFF_OF_D[d] holds the
    masks for key row r = 4j + d of pair j, cols rho*64 + w."""
    wm = np.zeros((128, 2, SLABW), np.float32)
    for d, base in OFF_OF_D.items():
        for rho in (0, 1):
            dh = d + 2 - 2 * rho
            if not 0 <= dh < 5:
                continue
            for w in range(64):
                for dw in range(5):
                    kc = 2 * w + dw - 2
                    if 0 <= kc < 128:
                        wm[kc, :, base + rho * 64 + w] = wpos[:, dh * 5 + dw]
    return wm


def make_oob():
    """#window entries with out-of-image column, per position in a pair."""
    oob = np.zeros((128, 1), np.float32)
    for rho in (0, 1):
        for w in range(64):
            cnt = sum(1 for dw in range(5) if not 0 <= 2 * w + dw - 2 < 128)
            oob[rho * 64 + w, 0] = 5.0 * cnt
    return oob


def _ap(t, off, dims, p0=0, pn=None):
    a = t[:]
    np_ = pn if pn is not None else a.ap[0][1]
    return bass.AP(tensor=a.tensor, offset=off + p0 * a.ap[0][0],
                   ap=[[a.ap[0][0], np_]] + [list(d) for d in dims])


def build_nc():
    nc = bacc.Bacc("TRN2", target_bir_lowering=False, debug=False, num_devices=NCORES)

    xe_d = nc.dram_tensor("xe", [CIN, 64, IMG], BF16, kind="ExternalInput")
    xo_d = nc.dram_tensor("xo", [CIN, 64, IMG], BF16, kind="ExternalInput")
    xte_d = nc.dram_tensor("xte", [128, XTROWS * XTW], BF16, kind="ExternalInput")
    g2_d = nc.dram_tensor("g2", [CIN, 128], BF16, kind="ExternalInput")
    wv_d = nc.dram_tensor("wve", [XTW, 258], BF16, kind="ExternalInput")
    wm_d = nc.dram_tensor("wmask", [128, 2 * SLABW], BF16, kind="ExternalInput")
    oob_d = nc.dram_tensor("oob", [128, 1], F32, kind="ExternalInput")
    out_d = nc.dram_tensor("out", [HO * HO, OC], BF16, kind="ExternalOutput")

    EXP = mybir.ActivationFunctionType.Exp

    with TileContext(nc) as tc, ExitStack() as ctx:
        sg = ctx.enter_context(tc.tile_pool(name="singles", bufs=1))
        # x: partitions 0:64 even image rows, 64:128 odd rows; 64 rows x 128 cols
        x_sb = sg.tile([128, 64 * IMG], BF16)
        xte_sb = sg.tile([128, XTROWS * XTW], BF16)
        y_sb = sg.tile([128, 4096], BF16)            # queries, dup on both halves
        wm_sb = sg.tile([128, 2 * SLABW], BF16)
        oob_sb = sg.tile([128, 1], F32)
        g2_sb = sg.tile([64, 128], BF16)
        wv_sb = sg.tile([XTW, 258], BF16)

        # sync queue: small critical constants first
        nc.sync.dma_start(out=g2_sb[:], in_=g2_d.ap())
        nc.sync.dma_start(out=wm_sb[:], in_=wm_d.ap())
        nc.sync.dma_start(out=wv_sb[:], in_=wv_d.ap())
        nc.sync.dma_start(out=oob_sb[:], in_=oob_d.ap())

        # gpsimd queue (cheap 25ns issue): x and xTe chunks, deadline order.
        # E/O chunk k: 8 packed rows (img rows 16k..16k+15), needed by slab 4k.
        # xte chunk c: 22 xte-rows, needed by pair ~(22c-8)/4.
        def x_chunk(c8):
            dst_e = _ap(x_sb, c8 * 8 * IMG, [[1, 8 * IMG]], 0, 64)
            dst_o = _ap(x_sb, c8 * 8 * IMG, [[1, 8 * IMG]], 64, 64)
            nc.gpsimd.dma_start(out=dst_e, in_=xe_d.ap()[:, c8 * 8:(c8 + 1) * 8, :])
            nc.gpsimd.dma_start(out=dst_o, in_=xo_d.ap()[:, c8 * 8:(c8 + 1) * 8, :])

        def xte_chunk(c):
            r0 = c * 22
            dst = _ap(xte_sb, r0 * XTW, [[1, 22 * XTW]])
            nc.gpsimd.dma_start(out=dst, in_=xte_d.ap()[:, r0 * XTW:(r0 + 22) * XTW])

        for k in range(8):
            x_chunk(k)
            if k < 6:
                xte_chunk(k)

        def xrow(r):
            # key row r: [64 partitions (channels), 128 cols] on its parity half
            p = (r & 1) * 64
            return x_sb[p:p + 64, (r >> 1) * IMG:(r >> 1) * IMG + IMG]

        with tc.tile_pool(name="stp", bufs=2, space="PSUM") as stpool, \
             tc.tile_pool(name="ybuf", bufs=1, space="PSUM") as ybuf, \
             tc.tile_pool(name="qp", bufs=1, space="PSUM") as qpool, \
             tc.tile_pool(name="apb", bufs=2, space="PSUM") as apool, \
             tc.tile_pool(name="e2t", bufs=3) as e2t, \
             tc.tile_pool(name="a0p", bufs=6) as a0p, \
             tc.tile_pool(name="qsb", bufs=3) as qsb, \
             tc.tile_pool(name="outsb", bufs=3) as outsb, \
             tc.tile_pool(name="dens", bufs=4) as dens:

            A = {}       # slab -> a01 tile [128, 2*SLABW]
            QS = {}      # pair -> qs tile [65, 256]
            AP2 = {}     # pair-group t -> apool tile [128, 258]

            def make_y(k):
                # y chunk k: queries for strided rows 8k..8k+8 -> y_sb cols
                # k*512..(k+1)*512; needs x even chunk k only.
                yp = ybuf.tile([128, 512], F32)
                rhs = _ap(x_sb, k * 8 * IMG, [[IMG, 8], [2, 64]], 0, 64)
                nc.tensor.matmul(yp[:], g2_sb[:], rhs, start=True, stop=True)
                dst = y_sb[:, k * 512:(k + 1) * 512]
                if k % 2 == 0:
                    nc.scalar.copy(dst, yp[:])
                else:
                    nc.vector.tensor_copy(dst, yp[:])

            def make_slab(s):
                stp = stpool.tile([128, 1024], F32)
                n = min(2, NPAIR - s) * 128
                # t=0 (tile0, bank A): pairs s-1, s -> cols 0:256
                if s == 0:
                    nc.tensor.matmul(stp[:, 128:256], xrow(0),
                                     y_sb[0:64, 0:128], start=True, stop=True)
                else:
                    nc.tensor.matmul(stp[:, 0:256], xrow(4 * s),
                                     y_sb[0:64, (s - 1) * 128:(s + 1) * 128],
                                     start=True, stop=True)
                # t=1 (tile1, bank B): pair s -> cols 512:640
                nc.tensor.matmul(stp[:, 512:640], xrow(4 * s + 1),
                                 y_sb[64:128, s * 128:(s + 1) * 128],
                                 start=True, stop=True)
                # t=2 (tile0, bank A): pairs s, s+1 -> cols 256:256+n
                nc.tensor.matmul(stp[:, 256:256 + n], xrow(4 * s + 2),
                                 y_sb[0:64, s * 128:s * 128 + n],
                                 start=True, stop=True)
                # t=3 (tile1, bank B): pairs s, s+1 -> cols 640:640+n
                nc.tensor.matmul(stp[:, 640:640 + n], xrow(4 * s + 3),
                                 y_sb[64:128, s * 128:s * 128 + n],
                                 start=True, stop=True)
                e2 = e2t.tile([128, SLABW], BF16)
                nc.scalar.activation(out=e2[:], in_=stp[:, 0:SLABW], func=EXP)
                a01 = a0p.tile([128, 2 * SLABW], BF16)
                e2r = bass.AP(tensor=e2[:].tensor, offset=e2[:].offset,
                              ap=[list(e2[:].ap[0]), [0, 2], [1, SLABW]])
                nc.vector.tensor_mul(a01[:], e2r, wm_sb[:])
                A[s] = a01
                A.pop(s - (DELAY_A + 2), None)

            def stage_a(j):
                # Q_m[c,pos] = sum_d sum_kc xTe_r[kc,c] a_m[kc,pos]; Q row 64
                # accumulates den (ones channel / wm pad rows).
                qp = qpool.tile([65, 256], F32)
                for m in (0, 1):
                    ds = (0, 1, 2, -2, -1, 3, 4)
                    for idx, d in enumerate(ds):
                        r = 4 * j + d
                        lhsT = _ap(xte_sb, (r + 2) * XTW, [[1, XTW]])
                        if r < 0 or r >= IMG:
                            rhs = wm_sb
                        else:
                            rhs = A[r // 4]
                        off = OFF_OF_D[d] + m * SLABW
                        nc.tensor.matmul(qp[:, m * 128:(m + 1) * 128],
                                         lhsT, rhs[:, off:off + 128],
                                         start=idx == 0, stop=idx == len(ds) - 1,
                                         skip_group_check=True)
                qs = qsb.tile([65, 256], BF16)
                nc.scalar.copy(qs[:], qp[:])
                QS[j] = qs

            def stage_b(j):
                # out[pos,ch'] (+den in col 128) = sum_m wv_m^T Qs_m
                t = j // 2
                if j % 2 == 0:
                    AP2[t] = apool.tile([128, 258], F32, name="ap2")
                ap2 = AP2[t]
                qs = QS.pop(j)
                c0 = (j % 2) * 129
                for m in (0, 1):
                    nc.tensor.matmul(ap2[:, c0:c0 + 129],
                                     qs[0:XTW, m * 128:(m + 1) * 128],
                                     wv_sb[:, m * 129:(m + 1) * 129],
                                     start=m == 0, stop=m == 1,
                                     skip_group_check=True)

            def finish_group(t):
                # pairs 2t, 2t+1: den = col 128 + oob, rec = 1/den,
                # out = ap[:,0:128] * rec -> bf16 -> DMA (one per pair).
                ap2 = AP2.pop(t)
                den = dens.tile([128, 2], F32)
                oob_b = bass.AP(tensor=oob_sb[:].tensor, offset=oob_sb[:].offset,
                                ap=[list(oob_sb[:].ap[0]), [0, 2]])
                nc.vector.tensor_add(den[:], _ap(ap2, 128, [[129, 2]]), oob_b)
                rec = dens.tile([128, 2], F32)
                nc.vector.reciprocal(rec[:], den[:])
                o_sb = outsb.tile([128, 256], BF16)
                for p in (0, 1):
                    nc.vector.tensor_scalar_mul(
                        o_sb[:, p * 128:(p + 1) * 128],
                        ap2[:, p * 129:p * 129 + 128], rec[:, p:p + 1])
                    j = 2 * t + p
                    nc.sync.dma_start(out=out_d.ap()[j * 128:(j + 1) * 128, :],
                                      in_=o_sb[:, p * 128:(p + 1) * 128])

            # software-pipelined main loop
            make_y(0)
            make_y(1)
            for s in range(NPAIR):
                make_slab(s)
                if s >= 2 and s % 4 == 2 and 2 + s // 4 < 8:
                    make_y(2 + s // 4)
                if s >= DELAY_A:
                    stage_a(s - DELAY_A)
                if s >= DELAY_A + 1:
                    stage_b(s - DELAY_A - 1)
                if s >= DELAY_A + 2 and (s - DELAY_A) % 2 == 0:
                    finish_group((s - DELAY_A) // 2 - 1)
            for j in range(NPAIR - DELAY_A, NPAIR):
                stage_a(j)
                stage_b(j - 1)
                if j % 2 == 0:
                    finish_group(j // 2 - 1)
            stage_b(NPAIR - 1)
            finish_group(NPAIR // 2 - 1)

    nc.compile()
    return nc


_NC_CACHE = None


def kernel(x, w_q, w_k, w_v, row_emb, col_emb, mix_emb):
    global _NC_CACHE
    x = np.asarray(x, np.float32)
    w_q = np.asarray(w_q, np.float32)
    w_k = np.asarray(w_k, np.float32)
    w_v = np.asarray(w_v, np.float32)
    row_emb = np.asarray(row_emb, np.float32)
    col_emb = np.asarray(col_emb, np.float32)
    mix_emb = np.asarray(mix_emb, np.float32)

    G = (w_q.T @ w_k).astype(ml_dtypes.bfloat16)          # [64, 64]
    g2 = np.hstack([G, G])                                # [64, 128]
    wpos = make_wpos(row_emb, col_emb, mix_emb)
    wmask = make_masks(wpos).reshape(128, 2 * SLABW).astype(ml_dtypes.bfloat16)
    oob = make_oob()

    # wv_ext: [65, 258]; block m: rows 0:64 col j = w_v[2j+m, c]; row 64 is
    # the den pass-through (-> out col 128).
    wve = np.zeros((XTW, 258), np.float32)
    for m in (0, 1):
        wve[0:64, m * 129:m * 129 + 128] = w_v[m::2, :].T
        wve[64, m * 129 + 128] = 1.0
    wve = wve.astype(ml_dtypes.bfloat16)

    xb = x.astype(ml_dtypes.bfloat16)                      # [B, 64, 128, 128]
    xe = np.ascontiguousarray(xb[:, :, 0::2, :])           # [B, 64, 64, 128]
    xo = np.ascontiguousarray(xb[:, :, 1::2, :])

    # xTe: [B, imgcol 128, row 132, ch 65]; ones channel 64; pad rows 0/1 and
    # 130/131 are zero except the ones channel.
    xte = np.zeros((NCORES, 128, XTROWS, XTW), np.float32)
    xte[:, :, 2:130, 0:64] = x.transpose(0, 3, 2, 1)
    xte[:, :, :, 64] = 1.0
    xte = xte.reshape(NCORES, 128, XTROWS * XTW).astype(ml_dtypes.bfloat16)

    if _NC_CACHE is None:
        _NC_CACHE = build_nc()
    nc = _NC_CACHE

    in_maps = []
    for b in range(NCORES):
        in_maps.append({
            "xe": xe[b],
            "xo": xo[b],
            "xte": xte[b],
            "g2": g2,
            "wve": wve,
            "wmask": wmask,
            "oob": oob,
        })
    res = run_bass_kernel_spmd(nc, in_maps, core_ids=list(range(NCORES)))
    out = np.stack([res.results[b]["out"].astype(np.float32).T.reshape(OC, HO, HO)
                    for b in range(NCORES)])
    return out


# revision 15
# speedup vs baseline: 1.1051x; 1.0107x over previous
"""Trainium2 Bass kernel for nn_AttentionStem (5x5 local attention stem, stride 2).

Self-contained: hardcodes shapes B=8, CIN=64, H=W=128, OUT_CH=128, M=2, K=5.
Data-parallel over batch: one batch element per NeuronCore (8 cores).

Math (per batch):
  scores[k,(h,w)] = x_s(2h,2w)^T G x(p'_k),  G = w_q^T w_k   (q/k folded)
  a_m[key,pos]    = exp(score) * wpos_m(dh,dw) * band
  out[pos,ch]     = sum_m wv_m^T ( sum_d xTe_r(d)^T a_m ) / den   (V folded
                    through the apply: Q_m[c,pos] = sum_keys a_m x[c,key],
                    then out = wv_m^T Q_m -- the big V tensor is never
                    materialized, killing the PSUM->SBUF V copy.)

v2 layout:
  - x bf16, even image rows on partitions 0:64, odd on 64:128 (ST row-tiling).
  - xTe: x transposed to [imgcol, row, ch] + ones channel + 2 pad rows each
    side (host-prepared). stage-A lhsT; ones channel accumulates den in Q
    row 64; pad rows make OOB key rows contribute sum(wpos) to den only.
  - per-slab chain: ST mms -> ACT exp -> DVE mask -> (3 iters later)
    stage-A mms -> ACT Q-copy -> stage-B mms -> DVE den/recip/scale -> DMA.
  - out stored bf16, host casts to f32.
"""

import sys

for _p in ("/opt/pypackages", "/opt/trn_rl_repo"):
    if _p not in sys.path:
        sys.path.insert(0, _p)

from contextlib import ExitStack

import ml_dtypes
import numpy as np

import concourse.bacc as bacc
import concourse.bass as bass
import concourse.mybir as mybir
from concourse.bass_utils import run_bass_kernel_spmd
from concourse.tile import TileContext

F32 = mybir.dt.float32
BF16 = mybir.dt.bfloat16

NCORES = 8
CIN = 64
IMG = 128          # input H = W
OC = 128           # out channels
HO = 64            # output H = W
NPAIR = 32         # output row pairs
SLABW = 896        # trimmed transposed-score slab width
XTW = 65           # xTe row width: 64 channels + ones
XTROWS = 132       # 2 pad + 128 + 2 pad rows in xTe

# d (= key row r - 4j for pair j) -> col offset of its 128-col block in a slab.
# Blocks from even key rows (PE row-tile 0) sit in PSUM bank A (cols 0:512),
# odd-row blocks (tile 1) in bank B (cols 512:896) -- concurrent row-tiled
# matmuls writing the same PSUM bank hang TRN2.
OFF_OF_D = {4: 0, 0: 128, 2: 256, -2: 384, 1: 512, 3: 640, -1: 768}

DELAY_A = 3        # stage-A for pair s-DELAY_A is emitted in iteration s


def make_wpos(row_emb, col_emb, mix_emb):
    a = mix_emb.T.astype(np.float64) @ row_emb.astype(np.float64)  # [2,5]
    b = mix_emb.T.astype(np.float64) @ col_emb.astype(np.float64)  # [2,5]
    wp = a[:, :, None] + b[:, None, :]                             # [2,5,5]
    wp = wp - wp.max(axis=0, keepdims=True)
    e = np.exp(wp)
    wp = e / e.sum(axis=0, keepdims=True)
    return wp.reshape(2, 25).astype(np.float32)                    # [m, dh*5+dw]


def make_masks(wpos):
    """wpos-weighted band masks in the trimmed ST layout.

    Returns [128 (kcol), 2 (m), 896] f32; block at OFF_OF_D[d] holds the
    masks for key row r = 4j + d of pair j, cols rho*64 + w."""
    wm = np.zeros((128, 2, SLABW), np.float32)
    for d, base in OFF_OF_D.items():
        for rho in (0, 1):
            dh = d + 2 - 2 * rho
            if not 0 <= dh < 5:
                continue
            for w in range(64):
                for dw in range(5):
                    kc = 2 * w + dw - 2
                    if 0 <= kc < 128:
                        wm[kc, :, base + rho * 64 + w] = wpos[:, dh * 5 + dw]
    return wm


def make_oob():
    """#window entries with out-of-image column, per position in a pair."""
    oob = np.zeros((128, 1), np.float32)
    for rho in (0, 1):
        for w in range(64):
            cnt = sum(1 for dw in range(5) if not 0 <= 2 * w + dw - 2 < 128)
            oob[rho * 64 + w, 0] = 5.0 * cnt
    return oob


def _ap(t, off, dims, p0=0, pn=None):
    a = t[:]
    np_ = pn if pn is not None else a.ap[0][1]
    return bass.AP(tensor=a.tensor, offset=off + p0 * a.ap[0][0],
                   ap=[[a.ap[0][0], np_]] + [list(d) for d in dims])


def build_nc():
    nc = bacc.Bacc("TRN2", target_bir_lowering=False, debug=False, num_devices=NCORES)

    xe_d = nc.dram_tensor("xe", [CIN, 64, IMG], BF16, kind="ExternalInput")
    xo_d = nc.dram_tensor("xo", [CIN, 64, IMG], BF16, kind="ExternalInput")
    xte_d = nc.dram_tensor("xte", [128, XTROWS * XTW], BF16, kind="ExternalInput")
    g2_d = nc.dram_tensor("g2", [CIN, 128], BF16, kind="ExternalInput")
    wv_d = nc.dram_tensor("wve", [XTW, 258], BF16, kind="ExternalInput")
    # wmask + a trailing 128-col block whose kc=0 row holds the col-OOB count
    # (stage-A adds it to the den row via the pad-ones lhsT).
    wm_d = nc.dram_tensor("wmask", [128, 2 * SLABW + 128], BF16, kind="ExternalInput")
    out_d = nc.dram_tensor("out", [HO * HO, OC], BF16, kind="ExternalOutput")

    EXP = mybir.ActivationFunctionType.Exp

    with TileContext(nc) as tc, ExitStack() as ctx:
        sg = ctx.enter_context(tc.tile_pool(name="singles", bufs=1))
        # x: partitions 0:64 even image rows, 64:128 odd rows; 64 rows x 128 cols
        x_sb = sg.tile([128, 64 * IMG], BF16)
        xte_sb = sg.tile([128, XTROWS * XTW], BF16)
        y_sb = sg.tile([128, 4096], BF16)            # queries, dup on both halves
        wm_sb = sg.tile([128, 2 * SLABW + 128], BF16)
        g2_sb = sg.tile([64, 128], BF16)
        wv_sb = sg.tile([XTW, 258], BF16)
        scr_sb = sg.tile([64, 512], BF16)            # HAM warmup scratch

        # sync queue: small critical constants first
        nc.sync.dma_start(out=g2_sb[:], in_=g2_d.ap())
        nc.sync.dma_start(out=wm_sb[:], in_=wm_d.ap())
        nc.sync.dma_start(out=wv_sb[:], in_=wv_d.ap())
        nc.gpsimd.memset(scr_sb[:], 0.0)

        # gpsimd queue (cheap 25ns issue): x and xTe chunks, deadline order.
        # E/O chunk k: 8 packed rows (img rows 16k..16k+15), needed by slab 4k.
        # xte chunk c: 22 xte-rows, needed by pair ~(22c-8)/4.
        def x_chunk(c8):
            dst_e = _ap(x_sb, c8 * 8 * IMG, [[1, 8 * IMG]], 0, 64)
            dst_o = _ap(x_sb, c8 * 8 * IMG, [[1, 8 * IMG]], 64, 64)
            nc.gpsimd.dma_start(out=dst_e, in_=xe_d.ap()[:, c8 * 8:(c8 + 1) * 8, :])
            nc.gpsimd.dma_start(out=dst_o, in_=xo_d.ap()[:, c8 * 8:(c8 + 1) * 8, :])

        def xte_chunk(c):
            r0 = c * 22
            dst = _ap(xte_sb, r0 * XTW, [[1, 22 * XTW]])
            nc.gpsimd.dma_start(out=dst, in_=xte_d.ap()[:, r0 * XTW:(r0 + 22) * XTW])

        for k in range(8):
            x_chunk(k)
            if k < 6:
                xte_chunk(k)

        def xrow(r):
            # key row r: [64 partitions (channels), 128 cols] on its parity half
            p = (r & 1) * 64
            return x_sb[p:p + 64, (r >> 1) * IMG:(r >> 1) * IMG + IMG]

        with tc.tile_pool(name="stp", bufs=2, space="PSUM") as stpool, \
             tc.tile_pool(name="ybuf", bufs=1, space="PSUM") as ybuf, \
             tc.tile_pool(name="qp", bufs=1, space="PSUM") as qpool, \
             tc.tile_pool(name="apb", bufs=2, space="PSUM") as apool, \
             tc.tile_pool(name="e2t", bufs=3) as e2t, \
             tc.tile_pool(name="a0p", bufs=6) as a0p, \
             tc.tile_pool(name="qsb", bufs=3) as qsb, \
             tc.tile_pool(name="outsb", bufs=3) as outsb, \
             tc.tile_pool(name="dens", bufs=4) as dens:

            A = {}       # slab -> a01 tile [128, 2*SLABW]
            QS = {}      # pair -> qs tile [65, 256]
            AP2 = {}     # pair-group t -> apool tile [128, 258]

            # HAM warmup: ~3.4us of dummy matmuls (no DMA deps) so the PE
            # clock is at 8/8 when the first real matmul issues.
            wup = stpool.tile([128, 1024], F32, name="wup", tag="st")
            for w in range(8):
                nc.tensor.matmul(wup[:, (w % 2) * 512:(w % 2) * 512 + 512],
                                 scr_sb[0:64, 0:128], scr_sb[0:64, 0:512],
                                 start=True, stop=True, skip_group_check=True)

            def make_y(k):
                # y chunk k: queries for strided rows 8k..8k+8 -> y_sb cols
                # k*512..(k+1)*512; needs x even chunk k only.
                yp = ybuf.tile([128, 512], F32)
                rhs = _ap(x_sb, k * 8 * IMG, [[IMG, 8], [2, 64]], 0, 64)
                nc.tensor.matmul(yp[:], g2_sb[:], rhs, start=True, stop=True)
                dst = y_sb[:, k * 512:(k + 1) * 512]
                if k % 2 == 0:
                    nc.scalar.copy(dst, yp[:])
                else:
                    nc.vector.tensor_copy(dst, yp[:])

            def make_slab(s):
                stp = stpool.tile([128, 1024], F32, tag="st")
                n = min(2, NPAIR - s) * 128
                # t=0 (tile0, bank A): pairs s-1, s -> cols 0:256
                if s == 0:
                    nc.tensor.matmul(stp[:, 128:256], xrow(0),
                                     y_sb[0:64, 0:128], start=True, stop=True)
                else:
                    nc.tensor.matmul(stp[:, 0:256], xrow(4 * s),
                                     y_sb[0:64, (s - 1) * 128:(s + 1) * 128],
                                     start=True, stop=True)
                # t=1 (tile1, bank B): pair s -> cols 512:640
                nc.tensor.matmul(stp[:, 512:640], xrow(4 * s + 1),
                                 y_sb[64:128, s * 128:(s + 1) * 128],
                                 start=True, stop=True)
                # t=2 (tile0, bank A): pairs s, s+1 -> cols 256:256+n
                nc.tensor.matmul(stp[:, 256:256 + n], xrow(4 * s + 2),
                                 y_sb[0:64, s * 128:s * 128 + n],
                                 start=True, stop=True)
                # t=3 (tile1, bank B): pairs s, s+1 -> cols 640:640+n
                nc.tensor.matmul(stp[:, 640:640 + n], xrow(4 * s + 3),
                                 y_sb[64:128, s * 128:s * 128 + n],
                                 start=True, stop=True)
                e2 = e2t.tile([128, SLABW], BF16)
                nc.scalar.activation(out=e2[:], in_=stp[:, 0:SLABW], func=EXP)
                a01 = a0p.tile([128, 2 * SLABW], BF16)
                e2r = bass.AP(tensor=e2[:].tensor, offset=e2[:].offset,
                              ap=[list(e2[:].ap[0]), [0, 2], [1, SLABW]])
                nc.vector.tensor_mul(a01[:], e2r, wm_sb[:, 0:2 * SLABW])
                A[s] = a01
                A.pop(s - (DELAY_A + 2), None)

            def stage_a(j):
                # Q_m[c,pos] = sum_d sum_kc xTe_r[kc,c] a_m[kc,pos]; Q row 64
                # accumulates den (ones channel / wm pad rows). m=0 also adds
                # the col-OOB count block (pad-ones lhsT x oob block).
                qp = qpool.tile([65, 256], F32)
                pad0 = _ap(xte_sb, 0, [[1, XTW]])
                for m in (0, 1):
                    ds = (0, 1, 2, -2, -1, 3, 4)
                    if m == 0:
                        nc.tensor.matmul(qp[:, 0:128], pad0,
                                         wm_sb[:, 2 * SLABW:2 * SLABW + 128],
                                         start=True, stop=False,
                                         skip_group_check=True)
                    for idx, d in enumerate(ds):
                        r = 4 * j + d
                        lhsT = _ap(xte_sb, (r + 2) * XTW, [[1, XTW]])
                        if r < 0 or r >= IMG:
                            rhs = wm_sb
                        else:
                            rhs = A[r // 4]
                        off = OFF_OF_D[d] + m * SLABW
                        nc.tensor.matmul(qp[:, m * 128:(m + 1) * 128],
                                         lhsT, rhs[:, off:off + 128],
                                         start=(idx == 0 and m == 1),
                                         stop=idx == len(ds) - 1,
                                         skip_group_check=True)
                qs = qsb.tile([65, 256], BF16)
                nc.scalar.copy(qs[:], qp[:])
                QS[j] = qs

            def stage_b(j):
                # out[pos,ch'] (+den in col 128) = sum_m wv_m^T Qs_m
                t = j // 2
                if j % 2 == 0:
                    AP2[t] = apool.tile([128, 258], F32, name="ap2")
                ap2 = AP2[t]
                qs = QS.pop(j)
                c0 = (j % 2) * 129
                for m in (0, 1):
                    nc.tensor.matmul(ap2[:, c0:c0 + 129],
                                     qs[0:XTW, m * 128:(m + 1) * 128],
                                     wv_sb[:, m * 129:(m + 1) * 129],
                                     start=m == 0, stop=m == 1,
                                     skip_group_check=True)

            def finish_group(t, only_p=None):
                # pairs 2t, 2t+1: rec = 1/den (den already includes oob via
                # stage-A), out = ap[:,0:128] * rec -> bf16 -> DMA per pair.
                last = only_p is not None
                if last and only_p == 0:
                    ap2 = AP2[t]
                else:
                    ap2 = AP2.pop(t)
                ps = (only_p,) if last else (0, 1)
                rec = dens.tile([128, 2], F32)
                nc.vector.reciprocal(rec[:, ps[0]:ps[-1] + 1],
                                     _ap(ap2, 128 + ps[0] * 129,
                                         [[129, len(ps)]]))
                o_sb = outsb.tile([128, 256], BF16)
                rec_b = bass.AP(tensor=rec[:].tensor,
                                offset=rec[:].offset + ps[0],
                                ap=[list(rec[:].ap[0]), [1, len(ps)], [0, 128]])
                src = bass.AP(tensor=ap2[:].tensor,
                              offset=ap2[:].offset + ps[0] * 129,
                              ap=[list(ap2[:].ap[0]), [129, len(ps)], [1, 128]])
                nc.vector.tensor_mul(
                    o_sb[:, ps[0] * 128:(ps[-1] + 1) * 128], src, rec_b)
                for p in ps:
                    j = 2 * t + p
                    q = nc.gpsimd if last else nc.sync
                    q.dma_start(out=out_d.ap()[j * 128:(j + 1) * 128, :],
                                in_=o_sb[:, p * 128:(p + 1) * 128])

            # software-pipelined main loop
            make_y(0)
            make_y(1)
            for s in range(NPAIR):
                make_slab(s)
                if s >= 2 and s % 4 == 2 and 2 + s // 4 < 8:
                    make_y(2 + s // 4)
                if s >= DELAY_A:
                    stage_a(s - DELAY_A)
                if s >= DELAY_A + 1:
                    stage_b(s - DELAY_A - 1)
                if s >= DELAY_A + 2 and (s - DELAY_A) % 2 == 0:
                    finish_group((s - DELAY_A) // 2 - 1)
            for j in range(NPAIR - DELAY_A, NPAIR):
                stage_a(j)
                stage_b(j - 1)
                if j % 2 == 0:
                    finish_group(j // 2 - 1)
            finish_group(NPAIR // 2 - 1, only_p=0)
            stage_b(NPAIR - 1)
            finish_group(NPAIR // 2 - 1, only_p=1)

    nc.compile()
    return nc


_NC_CACHE = None


def kernel(x, w_q, w_k, w_v, row_emb, col_emb, mix_emb):
    global _NC_CACHE
    x = np.asarray(x, np.float32)
    w_q = np.asarray(w_q, np.float32)
    w_k = np.asarray(w_k, np.float32)
    w_v = np.asarray(w_v, np.float32)
    row_emb = np.asarray(row_emb, np.float32)
    col_emb = np.asarray(col_emb, np.float32)
    mix_emb = np.asarray(mix_emb, np.float32)

    G = (w_q.T @ w_k).astype(ml_dtypes.bfloat16)          # [64, 64]
    g2 = np.hstack([G, G])                                # [64, 128]
    wpos = make_wpos(row_emb, col_emb, mix_emb)
    wmask = np.zeros((128, 2 * SLABW + 128), np.float32)
    wmask[:, 0:2 * SLABW] = make_masks(wpos).reshape(128, 2 * SLABW)
    wmask[0, 2 * SLABW:] = make_oob()[:, 0]
    wmask = wmask.astype(ml_dtypes.bfloat16)

    # wv_ext: [65, 258]; block m: rows 0:64 col j = w_v[2j+m, c]; row 64 is
    # the den pass-through (-> out col 128).
    wve = np.zeros((XTW, 258), np.float32)
    for m in (0, 1):
        wve[0:64, m * 129:m * 129 + 128] = w_v[m::2, :].T
        wve[64, m * 129 + 128] = 1.0
    wve = wve.astype(ml_dtypes.bfloat16)

    xb = x.astype(ml_dtypes.bfloat16)                      # [B, 64, 128, 128]
    xe = np.ascontiguousarray(xb[:, :, 0::2, :])           # [B, 64, 64, 128]
    xo = np.ascontiguousarray(xb[:, :, 1::2, :])

    # xTe: [B, imgcol 128, row 132, ch 65]; ones channel 64; pad rows 0/1 and
    # 130/131 are zero except the ones channel.
    xte = np.zeros((NCORES, 128, XTROWS, XTW), np.float32)
    xte[:, :, 2:130, 0:64] = x.transpose(0, 3, 2, 1)
    xte[:, :, :, 64] = 1.0
    xte = xte.reshape(NCORES, 128, XTROWS * XTW).astype(ml_dtypes.bfloat16)

    if _NC_CACHE is None:
        _NC_CACHE = build_nc()
    nc = _NC_CACHE

    in_maps = []
    for b in range(NCORES):
        in_maps.append({
            "xe": xe[b],
            "xo": xo[b],
            "xte": xte[b],
            "g2": g2,
            "wve": wve,
            "wmask": wmask,
        })
    res = run_bass_kernel_spmd(nc, in_maps, core_ids=list(range(NCORES)))
    out = np.stack([res.results[b]["out"].astype(np.float32).T.reshape(OC, HO, HO)
                    for b in range(NCORES)])
    return out
